# revision 8
# baseline (speedup 1.0000x reference)
"""GCN layer (nn_GCNLayer_901943132166) on 8 Trainium2 NeuronCores.

Strategy: partition dst nodes across 8 cores (1D). Host sorts each core's
edges by dst super-block (512 dst nodes), groups them by src range (int16
gather index limit = 32768 rows), and ships per-edge metadata. On device:
dma_gather edge source rows, build a norm-scaled one-hot S [edge, dst] with
one fused DVE tensor_scalar (iota == dst_local) * norm, accumulate
aggT = E^T @ S in PSUM per super-block, then agg @ W + bias via a second
matmul. Both degree norms are folded into the per-edge scale on the host
(pure index-derived metadata).
"""
import os
import sys

import numpy as np

N_NODES = 100000
N_EDGES = 1600000
F = 128            # feature dim (in == out)
N_CORES = 8
OWN = 12544        # dst nodes owned per core (98 * 128)
SB = 512           # dst super-block width (one PSUM bank of f32)
NSB = 25           # super-blocks per core (25*512 = 12800 >= 12544)
RANGE = 32768      # int16 gather index range
NRANGES = 4        # ceil(100000 / 32768)
MAX_CALL = 896     # dma_gather single-packet cap (64 descs/ring incl. sem)


def _install_walrus_passes():
    """This walrus build omits the dynamic-DMA passes that set up the SWDGE
    descriptor rings dma_gather needs; splice them into the pass list."""
    import concourse.bass_utils as bu

    def patched(tmpdir, inp="bir.json", outp="file.neff", arch=None, *, dve_root=None):
        from pathlib import Path
        cmd = [
            bu.get_walrus_driver(),
            "--pass",
            "birverifier,dynamic_dma_scan,runtime_memory_reservation,"
            "dynamic_dma_setup,lower_act,lower_dve,lower_ap_offset,"
            "codegen,neff_packager",
            "-i", inp,
            "--neff-output-filename", outp,
            "--enable-birsim=true",
            "--mem-mode=physical",
            "--policy=0",
            "--enable-ldw-opt=false",
            "--assign-static-dmas-to-sp=false",
            "--dram-page-size=256",
            "--enable-neff-debug-info=true",
            "--jobs", "8",
            "--dynamic-dma-scratch-size-per-partition=16384",
            *bu.get_walrus_args(
                bu.get_bir_arch(tmpdir, inp) if arch is None else arch,
                tmpdir, dve_root=dve_root,
            ),
        ]
        result = bu.run_command(cmd, cwd=tmpdir)
        if result is not None:
            (Path(tmpdir) / "log.txt").write_text(result.stdout)
        return f"{tmpdir}/{outp}"

    bu.bir_verify_and_optimise = patched


def _pack_idx_wrap(idx_i16: np.ndarray, cap: int) -> np.ndarray:
    """int16 idx buffer [128, cap//16]: idx j -> [j%16, j//16], replicated
    across the eight 16-partition groups (Q7 core pairs read different
    partition windows)."""
    w = np.zeros((16, cap // 16), np.int16)
    j = np.arange(len(idx_i16))
    w[j % 16, j // 16] = idx_i16
    return np.tile(w, (8, 1))


def _preprocess(src: np.ndarray, dst: np.ndarray):
    """Host-side index marshaling. Returns the static call plan (shared by
    all cores) and per-core data arrays."""
    src = np.asarray(src).astype(np.int64)
    dst = np.asarray(dst).astype(np.int64)

    ones = np.ones(len(src), np.float32)
    out_deg = np.bincount(src, minlength=N_NODES).astype(np.float32)
    in_deg = np.bincount(dst, minlength=N_NODES).astype(np.float32)
    norm_src = 1.0 / np.sqrt(np.clip(out_deg, 1.0, None))
    norm_dst = 1.0 / np.sqrt(np.clip(in_deg, 1.0, None))
    norm_edge = (norm_src[src] * norm_dst[dst]).astype(np.float32)

    core = np.minimum(dst // OWN, N_CORES - 1)
    dst_local = dst - core * OWN
    sb = dst_local // SB
    rng = src // RANGE

    # group sizes per (core, sb, range)
    sizes = np.zeros((N_CORES, NSB, NRANGES), np.int64)
    np.add.at(sizes, (core, sb, rng), 1)
    gmax = sizes.max(axis=0)                       # [NSB, NRANGES]
    gpad = ((gmax + 127) // 128) * 128             # padded group capacity
    gpad = np.maximum(gpad, 0)

    # static call plan: per (sb, r) a list of call sizes (multiples of 128,
    # each <= MAX_CALL)
    plan = []          # (sb, r, call_cols_offset_chunks, n_idx)
    chunk_of = []      # per chunk: nothing needed beyond order
    total_chunks = 0
    for s in range(NSB):
        for r in range(NRANGES):
            n = int(gpad[s, r])
            if n == 0:
                continue
            off = 0
            while off < n:
                take = min(MAX_CALL, n - off)
                plan.append((s, r, total_chunks, take))
                total_chunks += take // 128
                off += take

    chunks_per_sb = np.zeros(NSB, np.int64)
    for s, r, c0, n in plan:
        chunks_per_sb[s] += n // 128

    # order edges per core: by (sb, range), then pad groups
    idx_cols = total_chunks * 8                    # int16 cols per core ([128, cols])
    per_core = []
    for k in range(N_CORES):
        m = core == k
        e_sb, e_rng = sb[m], rng[m]
        e_src, e_dstl, e_norm = src[m], dst_local[m], norm_edge[m]
        order = np.lexsort((e_rng, e_sb))
        e_sb, e_rng = e_sb[order], e_rng[order]
        e_src, e_dstl, e_norm = e_src[order], e_dstl[order], e_norm[order]

        # build padded streams
        idx_stream = np.zeros(total_chunks * 128, np.int16)
        dloc_stream = np.zeros(total_chunks * 128, np.float32)
        norm_stream = np.zeros(total_chunks * 128, np.float32)
        # group start offsets in the sorted arrays
        gsizes = np.zeros((NSB, NRANGES), np.int64)
        np.add.at(gsizes, (e_sb, e_rng), 1)
        gstart = {}
        acc = 0
        for s in range(NSB):
            for r in range(NRANGES):
                gstart[(s, r)] = acc
                acc += int(gsizes[s, r])

        pos = 0  # position in padded stream (edges)
        for s in range(NSB):
            for r in range(NRANGES):
                n_real = int(gsizes[s, r])
                capn = int(gpad[s, r])
                if capn == 0:
                    continue
                a = gstart[(s, r)]
                sl = slice(pos, pos + n_real)
                idx_stream[sl] = (e_src[a:a + n_real] - r * RANGE).astype(np.int16)
                dloc_stream[sl] = (e_dstl[a:a + n_real] - s * SB).astype(np.float32)
                norm_stream[sl] = e_norm[a:a + n_real]
                # padding: idx 0 (valid row), norm 0 -> contributes nothing
                pos += capn
        assert pos == total_chunks * 128

        # pack idx per call into the wrap layout, concatenated column-wise
        idx_buf = np.zeros((128, idx_cols), np.int16)
        for s, r, c0, n in plan:
            seg = idx_stream[c0 * 128: c0 * 128 + n]
            idx_buf[:, c0 * 8: c0 * 8 + n // 16] = _pack_idx_wrap(seg, n)

        meta = np.stack([
            dloc_stream.reshape(total_chunks, 128),
            norm_stream.reshape(total_chunks, 128),
        ], axis=1)                                  # [chunks, 2, 128] f32
        per_core.append((idx_buf, meta.astype(np.float32)))

    return plan, chunks_per_sb, total_chunks, idx_cols, per_core


def _build_program(plan, chunks_per_sb, total_chunks, idx_cols):
    import concourse.bacc as bacc
    import concourse.mybir as mybir
    import concourse.tile as tile

    nc = bacc.Bacc()
    feat_d = nc.declare_dram_parameter("feat", [N_NODES, F], mybir.dt.float32, isOutput=False)
    w_d = nc.declare_dram_parameter("w", [F, F], mybir.dt.float32, isOutput=False)
    bias_d = nc.declare_dram_parameter("biasb", [128, SB], mybir.dt.float32, isOutput=False)
    iota_d = nc.declare_dram_parameter("iota", [128, SB], mybir.dt.float16, isOutput=False)
    idx_d = nc.declare_dram_parameter("idxb", [128, idx_cols], mybir.dt.int16, isOutput=False)
    meta_d = nc.declare_dram_parameter("meta", [total_chunks, 2, 128], mybir.dt.float32, isOutput=False)
    out_d = nc.declare_dram_parameter("out", [NSB * SB, F], mybir.dt.float32, isOutput=True)

    ranges = [(r * RANGE, min((r + 1) * RANGE, N_NODES)) for r in range(NRANGES)]

    with tile.TileContext(nc) as tc:
        with (
            tc.tile_pool(name="const", bufs=1) as constp,
            tc.tile_pool(name="et", bufs=3) as etp,
            tc.tile_pool(name="ix", bufs=4) as ixp,
            tc.tile_pool(name="mt", bufs=3) as mtp,
            tc.tile_pool(name="s", bufs=6) as sp,
            tc.tile_pool(name="aggs", bufs=2) as aggsp,
            tc.tile_pool(name="outs", bufs=2) as outsp,
            tc.tile_pool(name="ps", bufs=2, space="PSUM") as psp,
            tc.tile_pool(name="ps2", bufs=2, space="PSUM") as ps2p,
        ):
            w_t = constp.tile([F, F], mybir.dt.float32)
            nc.sync.dma_start(w_t[:], w_d[:])
            bias_t = constp.tile([128, SB], mybir.dt.float32)
            nc.sync.dma_start(bias_t[:], bias_d[:])
            iota_t = constp.tile([128, SB], mybir.dt.float16)
            nc.sync.dma_start(iota_t[:], iota_d[:])

            sb_plan = {}
            for s, r, c0, n in plan:
                sb_plan.setdefault(s, []).append((r, c0, n))

            chunk_base = 0
            for s in range(NSB):
                nch = int(chunks_per_sb[s])
                if nch == 0:
                    continue
                calls = sb_plan[s]
                # edge features for the whole super-block
                et = etp.tile([128, nch * F], mybir.dt.float32)
                for r, c0, n in calls:
                    lo, hi = ranges[r]
                    ix = ixp.tile([128, idx_cols and (MAX_CALL // 16)], mybir.dt.int16, tag="ix")
                    nc.sync.dma_start(ix[:, : n // 16], idx_d[:, c0 * 8: c0 * 8 + n // 16])
                    rel = c0 - chunk_base
                    nc.gpsimd.dma_gather(
                        out_ap=et[:, rel * F: (rel + n // 128) * F].rearrange(
                            "p (c e) -> p c e", e=F),
                        in_ap=feat_d[lo:hi, :],
                        idxs_ap=ix[:, : n // 16],
                        num_idxs=n,
                        num_idxs_reg=n,
                        elem_size=F,
                    )
                # per-chunk metadata [128, 2*nch]
                mt = mtp.tile([128, 2 * nch], mybir.dt.float32)
                nc.sync.dma_start(
                    mt[:],
                    meta_d[chunk_base: chunk_base + nch].rearrange("c t p -> p (c t)"),
                )
                # accumulate aggT [f, dst] over chunks
                psT = psp.tile([128, SB], mybir.dt.float32, space="PSUM")
                for c in range(nch):
                    st = sp.tile([128, SB], mybir.dt.float32, tag="s")
                    if c % 3 == 0:
                        # fused on DVE: keeps the otherwise-loaded ACT engine free
                        nc.vector.tensor_scalar(
                            out=st[:],
                            in0=iota_t[:],
                            scalar1=mt[:, 2 * c: 2 * c + 1],
                            scalar2=mt[:, 2 * c + 1: 2 * c + 2],
                            op0=mybir.AluOpType.is_equal,
                            op1=mybir.AluOpType.mult,
                        )
                    else:
                        s01 = sp.tile([128, SB], mybir.dt.float16, tag="s01")
                        nc.vector.tensor_scalar(
                            out=s01[:],
                            in0=iota_t[:],
                            scalar1=mt[:, 2 * c: 2 * c + 1],
                            scalar2=None,
                            op0=mybir.AluOpType.is_equal,
                        )
                        nc.scalar.activation(
                            st[:], s01[:], mybir.ActivationFunctionType.Copy,
                            scale=mt[:, 2 * c + 1: 2 * c + 2],
                        )
                    nc.tensor.matmul(
                        out=psT[:],
                        lhsT=et[:, c * F: (c + 1) * F],
                        rhs=st[:],
                        start=(c == 0),
                        stop=(c == nch - 1),
                    )
                aggT = aggsp.tile([128, SB], mybir.dt.float32)
                nc.scalar.copy(aggT[:], psT[:])
                ps2 = ps2p.tile([128, SB], mybir.dt.float32, space="PSUM")
                for j in range(SB // F):
                    nc.tensor.matmul(
                        out=ps2[:, j * F: (j + 1) * F],
                        lhsT=aggT[:, j * F: (j + 1) * F],
                        rhs=w_t[:],
                        start=True,
                        stop=True,
                    )
                ot = outsp.tile([128, SB], mybir.dt.float32)
                nc.vector.tensor_add(ot[:], ps2[:], bias_t[:])
                nc.sync.dma_start(
                    out_d[s * SB: (s + 1) * SB, :].rearrange("(j p) f -> p j f", p=128),
                    ot[:].rearrange("p (j f) -> p j f", f=F),
                )
                chunk_base += nch
    nc.finalize()
    return nc


def kernel(feat, weight, bias, src, dst):
    _install_walrus_passes()
    from concourse.bass_utils import run_bass_kernel_spmd

    feat = np.ascontiguousarray(np.asarray(feat, dtype=np.float32))
    weight = np.ascontiguousarray(np.asarray(weight, dtype=np.float32))
    bias = np.asarray(bias, dtype=np.float32)

    plan, chunks_per_sb, total_chunks, idx_cols, per_core = _preprocess(src, dst)
    nc = _build_program(plan, chunks_per_sb, total_chunks, idx_cols)

    bias_b = np.broadcast_to(np.tile(bias, SB // F)[None, :], (128, SB)).copy()
    iota = np.broadcast_to(np.arange(SB, dtype=np.float16)[None, :], (128, SB)).copy()

    in_maps = []
    for k in range(N_CORES):
        idx_buf, meta = per_core[k]
        in_maps.append({
            "feat": feat,
            "w": weight,
            "biasb": bias_b,
            "iota": iota,
            "idxb": idx_buf,
            "meta": meta,
        })
    res = run_bass_kernel_spmd(nc, in_maps, list(range(N_CORES)))
    out = np.empty((N_CORES * OWN, F), np.float32)
    for k in range(N_CORES):
        out[k * OWN: (k + 1) * OWN] = res.results[k]["out"][:OWN]
    return out[:N_NODES]
